# revision 7
# baseline (speedup 1.0000x reference)
"""Trainium2 Bass kernel for the BaselinePreprocessor problem.

Computes, for full inputs:
  fused = concat([interp(vision->T), interp(proprio->T), imu], -1)  # [64,1024,550]
  vox_mean = mean(occupancy grid 64^3 of 10k points)               # scalar
  out = concat([fused, vox_mean bcast], -1)                        # [64,1024,551]

Strategy (v3):
- Pure data parallel over batch (8 cores x 8 batches).
- Interp along time is a dense fp16 TensorE matmul with host-precomputed
  weights (one matmul per time tile, tolerance 2e-2 >> fp16 error ~1e-3).
- Inputs are host-transposed/cast so every DMA has large contiguous
  descriptors (imu -> [T,B,6] gives 192B rows instead of 24B).
- Output written as ONE DMA per time tile covering all 8 batches
  ([128, 8, 550/551] -> 2200B descriptors), alternating sync/scalar queues.
- Voxel occupancy without DRAM scatter: points are replicated to all
  cores; core i counts distinct voxels in slab [i*32768,(i+1)*32768) via
  bf16 one-hot is_equal tiles + 79 accumulating matmuls into one PSUM
  tile (count[hi,lo] += [hi(idx_k)==hi]*[lo(idx_k)==lo]), then a Sign
  activation with accum_out clamps+row-sums, and a tiny AllReduce(add)
  sums the disjoint slab counts. The voxel chain is emitted FIRST so the
  collective's ~40us fixed latency overlaps the output stream; tiles
  j >= J0 get the vox column inline, earlier tiles get column patch DMAs.
"""

import numpy as np

import concourse.bacc as bacc
import concourse.bass as bass
import concourse.mybir as mybir
import concourse.tile as tile
from concourse.bass_utils import run_bass_kernel_spmd

F32 = mybir.dt.float32
F16 = mybir.dt.float16
BF16 = mybir.dt.bfloat16
I32 = mybir.dt.int32
ALU = mybir.AluOpType
ACT = mybir.ActivationFunctionType

N_CORES = 8
B = 8                      # batches per core
T = 1024
LV, CV = 64, 512           # vision input time-len, channels
LP, CP = 256, 32           # proprio
CI = 6                     # imu channels (identity interp: L == T)
C_OUT = 551
GRID = 64
NVOX = GRID * GRID * GRID  # 262144
NPTS = 10000
PTS_F = 79                 # points laid out [128, 79] (padded to 10112)
NPTS_PAD = 128 * PTS_F
SLAB = NVOX // N_CORES     # 32768 voxels per core's slab
HI, LO = 128, 256          # slab voxel index split: idx_local = hi*256 + lo
N_TILES = T // 128         # 8 time tiles of 128 rows
OH_CHUNKS = [20, 20, 20, 19]  # point-column chunking for one-hot builds
J0 = 6                     # tiles >= J0 write the vox column inline


def _interp_weights_T(L: int) -> np.ndarray:
    """W^T [L, T] with W the [T, L] linear-interp matrix (align_corners)."""
    scale = np.float32((L - 1) / (T - 1))
    pos = np.arange(T, dtype=np.float32) * scale
    lo = np.clip(np.floor(pos).astype(np.int32), 0, L - 1)
    hi = np.minimum(lo + 1, L - 1)
    w = (pos - lo.astype(np.float32)).astype(np.float32)
    wt = np.zeros((L, T), dtype=np.float32)
    np.add.at(wt, (lo, np.arange(T)), np.float32(1.0) - w)
    np.add.at(wt, (hi, np.arange(T)), w)
    return np.ascontiguousarray(wt)


def _proprio_chunks_needed(j: int) -> list[int]:
    """Which K=128 row chunks of W_p^T have nonzeros for time tile j."""
    lo0 = (128 * j * (LP - 1)) // (T - 1)
    lo1 = (128 * j + 127) * (LP - 1) // (T - 1)
    hi1 = min(lo1 + 1, LP - 1)
    ks = []
    if lo0 < 128:
        ks.append(0)
    if hi1 >= 128:
        ks.append(1)
    return ks


def _emit(nc: bass.Bass, tc: tile.TileContext, ctx):
    vis = nc.declare_dram_parameter("vis", [LV, B, CV], F16, isOutput=False)
    pro = nc.declare_dram_parameter("pro", [LP, B, CP], F16, isOutput=False)
    imu = nc.declare_dram_parameter("imu", [T, B, CI], F32, isOutput=False)
    pts = nc.declare_dram_parameter("pts", [NPTS_PAD, 3], F32, isOutput=False)
    wv = nc.declare_dram_parameter("wv", [LV, T], F16, isOutput=False)
    wp = nc.declare_dram_parameter("wp", [LP, T], F16, isOutput=False)
    slab = nc.declare_dram_parameter("slab", [128, 1], I32, isOutput=False)
    out = nc.declare_dram_parameter("out", [B, T, C_OUT], F32, isOutput=True)

    cnt_dram = nc.dram_tensor("cnt", [1, 128], F32)
    cnt_sh = nc.dram_tensor("cnt_sh", [1, 128], F32, addr_space="Shared")

    const = ctx.enter_context(tc.tile_pool(name="const", bufs=1))
    vxw = ctx.enter_context(tc.tile_pool(name="vxw", bufs=1))
    ohp = ctx.enter_context(tc.tile_pool(name="ohp", bufs=2))
    rp = ctx.enter_context(tc.tile_pool(name="rp", bufs=2))
    outp = ctx.enter_context(tc.tile_pool(name="outp", bufs=3))
    psumv = ctx.enter_context(tc.tile_pool(name="psumv", bufs=3, space="PSUM"))
    psump = ctx.enter_context(tc.tile_pool(name="psump", bufs=2, space="PSUM"))
    psumg = ctx.enter_context(tc.tile_pool(name="psumg", bufs=1, space="PSUM"))
    psums = ctx.enter_context(tc.tile_pool(name="psums", bufs=1, space="PSUM"))

    # ---------------- loads (pts first: the voxel chain leads) ----------------
    pts_sb = vxw.tile([128, PTS_F, 3], F32)
    nc.sync.dma_start(out=pts_sb[:], in_=pts[:].rearrange("(p f) c -> p f c", p=128))
    slab_sb = vxw.tile([128, 1, 1], I32)
    nc.sync.dma_start(out=slab_sb[:, :, 0], in_=slab[:])
    wv_sb = const.tile([LV, T], F16)
    nc.scalar.dma_start(out=wv_sb[:], in_=wv[:])
    wp_sb = const.tile([128, 2, T], F16)
    nc.scalar.dma_start(out=wp_sb[:], in_=wp[:].rearrange("(k p) t -> p k t", p=128))
    vh_sb = const.tile([LV, B, CV], F16)
    nc.scalar.dma_start(out=vh_sb[:], in_=vis[:])
    pro_sb = const.tile([128, 2, B, CP], F16)
    nc.scalar.dma_start(out=pro_sb[:], in_=pro[:].rearrange("(k p) b c -> p k b c", p=128))
    imu_sb = const.tile([128, N_TILES, B, CI], F32)
    nc.scalar.dma_start(out=imu_sb[:], in_=imu[:].rearrange("(j p) b c -> p j b c", p=128))

    # iotas for the one-hot builds (bf16: integers <= 256 are exact)
    iota_hi_i = const.tile([128, 1, HI], I32)
    nc.gpsimd.iota(iota_hi_i[:], pattern=[[1, HI]], base=0, channel_multiplier=0)
    iota_lo_i = const.tile([128, 1, LO], I32)
    nc.gpsimd.iota(iota_lo_i[:], pattern=[[1, LO]], base=0, channel_multiplier=0)
    iota_hi = const.tile([128, 1, HI], BF16)
    nc.vector.tensor_copy(out=iota_hi[:], in_=iota_hi_i[:])
    iota_lo = const.tile([128, 1, LO], BF16)
    nc.vector.tensor_copy(out=iota_lo[:], in_=iota_lo_i[:])
    ones_col = const.tile([128, 1], F32)
    nc.gpsimd.memset(ones_col[:], 1.0)
    ones_row = const.tile([1, 128], F32)
    nc.gpsimd.memset(ones_row[:], 1.0)

    # ---------------- voxel index math (vector, leads the queue) ----------------
    # q_c = clip(trunc((p_c + 2) * 16), 0, 63), computed clip-then-floor
    # (equivalent: trunc==floor on the surviving non-negative range).
    # floor via int32 round-trip (any rounding mode) + is_gt correction.
    q = []
    ji = vxw.tile([128, PTS_F, 1], I32)
    gt = vxw.tile([128, PTS_F, 1], F32)
    for c in range(3):
        qc = vxw.tile([128, PTS_F, 1], F32, tag=f"q{c}")
        nc.vector.tensor_scalar(qc[:, :, 0], pts_sb[:, :, c], 2.0, 16.0, ALU.add, ALU.mult)
        nc.vector.tensor_scalar(qc[:], qc[:], 63.0, 0.0, ALU.min, ALU.max)
        rt = vxw.tile([128, PTS_F, 1], F32, tag=f"rt{c}")
        nc.vector.tensor_copy(out=ji[:], in_=qc[:])
        nc.vector.tensor_copy(out=rt[:], in_=ji[:])
        nc.vector.tensor_tensor(gt[:], rt[:], qc[:], ALU.is_gt)
        nc.vector.tensor_tensor(qc[:], rt[:], gt[:], ALU.subtract)
        q.append(qc)
    acc = vxw.tile([128, PTS_F, 1], F32)
    nc.vector.tensor_scalar(acc[:], q[0][:], 64.0, None, ALU.mult)
    nc.vector.tensor_tensor(acc[:], acc[:], q[1][:], ALU.add)
    nc.vector.tensor_scalar(acc[:], acc[:], 64.0, None, ALU.mult)
    nc.vector.tensor_tensor(acc[:], acc[:], q[2][:], ALU.add)
    idx_i = vxw.tile([128, PTS_F, 1], I32)
    nc.vector.tensor_copy(out=idx_i[:], in_=acc[:])  # exact integers -> exact
    # slab-local index; out-of-slab points self-mask (hi outside [0,128))
    nc.vector.tensor_tensor(
        idx_i[:], idx_i[:], slab_sb[:].to_broadcast([128, PTS_F, 1]), ALU.subtract
    )
    hi_i = vxw.tile([128, PTS_F, 1], I32)
    nc.vector.tensor_scalar(hi_i[:], idx_i[:], 8, None, ALU.arith_shift_right)
    lo_i = vxw.tile([128, PTS_F, 1], I32)
    nc.vector.tensor_scalar(lo_i[:], idx_i[:], 255, None, ALU.bitwise_and)
    # bf16 copies: slab-local hi in [0,128) / lo in [0,256) are exact;
    # out-of-slab hi only needs to stay outside [0,128), which rounding
    # preserves (relative error << distance to the valid range).
    hi_b = vxw.tile([128, PTS_F, 1], BF16)
    nc.vector.tensor_copy(out=hi_b[:], in_=hi_i[:])
    lo_b = vxw.tile([128, PTS_F, 1], BF16)
    nc.vector.tensor_copy(out=lo_b[:], in_=lo_i[:])

    # one-hot chunks (vector, right behind the index math)
    grid_ps = psumg.tile([128, LO], F32)
    n_chunks = len(OH_CHUNKS)
    chunk_start = [sum(OH_CHUNKS[:c]) for c in range(n_chunks)]
    oh_tiles = {}
    r_tiles = {}
    for c in range(n_chunks):
        f0, fn = chunk_start[c], OH_CHUNKS[c]
        oh = ohp.tile([128, max(OH_CHUNKS), HI], BF16, tag="oh")
        nc.vector.tensor_tensor(
            oh[:, 0:fn, :],
            hi_b[:, f0:f0 + fn, :].to_broadcast([128, fn, HI]),
            iota_hi[:].to_broadcast([128, fn, HI]),
            ALU.is_equal,
        )
        r = rp.tile([128, max(OH_CHUNKS), LO], BF16, tag="r")
        nc.vector.tensor_tensor(
            r[:, 0:fn, :],
            lo_b[:, f0:f0 + fn, :].to_broadcast([128, fn, LO]),
            iota_lo[:].to_broadcast([128, fn, LO]),
            ALU.is_equal,
        )
        oh_tiles[c] = oh
        r_tiles[c] = r

    def emit_grid_mms(c):
        f0, fn = chunk_start[c], OH_CHUNKS[c]
        for f in range(fn):
            nc.tensor.matmul(
                out=grid_ps[:],
                lhsT=oh_tiles[c][:, f, :],
                rhs=r_tiles[c][:, f, :],
                start=(f0 + f == 0),
                stop=(f0 + f == NPTS_PAD // 128 - 1),
            )

    # ---------------- proprio prepass: pp[j] for all batches ----------------
    pp_tiles = []
    for j in range(N_TILES):
        js = slice(j * 128, (j + 1) * 128)
        ppj = psump.tile([128, B, CP], F32, tag="pp")
        ks = _proprio_chunks_needed(j)
        for i, k in enumerate(ks):
            nc.tensor.matmul(
                out=ppj[:],
                lhsT=wp_sb[:, k, js],
                rhs=pro_sb[:, k, :, :],
                start=(i == 0),
                stop=(i == len(ks) - 1),
            )
        pp_sb = const.tile([128, B, CP], F32, tag=f"ppsb{j}", name=f"ppsb{j}")
        nc.vector.tensor_copy(out=pp_sb[:], in_=ppj[:])
        pp_tiles.append(pp_sb)

    # ---------------- voxel count -> AllReduce (emitted before the j loop
    # so the collective's fixed latency overlaps the stream) ----------------
    occ = vxw.tile([128, LO], BF16)
    red = vxw.tile([128, 1], F32)
    cnt_sb = vxw.tile([1, 128], F32)
    nc.gpsimd.memset(cnt_sb[:], 0.0)
    cnt_ps = psums.tile([1, 1], F32, tag="cnt")
    cnt_rb = vxw.tile([1, 128], F32)
    vox1 = vxw.tile([1, 1], F32)
    vox_pb = psums.tile([128, 1], F32, tag="voxb")
    vox_row = vxw.tile([128, B], F32)

    def emit_vox_tail():
        # Sign(count): counts >= 0 -> exactly the 0/1 occupancy; accum_out row-sums
        nc.scalar.activation(out=occ[:], in_=grid_ps[:], func=ACT.Sign, accum_out=red[:])
        nc.tensor.matmul(out=cnt_ps[:], lhsT=red[:], rhs=ones_col[:], start=True, stop=True)
        nc.scalar.activation(out=cnt_sb[:, 0:1], in_=cnt_ps[:], func=ACT.Copy)
        nc.gpsimd.dma_start(out=cnt_dram[:], in_=cnt_sb[:])
        nc.gpsimd.collective_compute(
            "AllReduce",
            ALU.add,
            replica_groups=[list(range(N_CORES))],
            ins=[cnt_dram[:]],
            outs=[cnt_sh[:]],
        )
        nc.gpsimd.dma_start(out=cnt_rb[:], in_=cnt_sh[:])
        nc.gpsimd.tensor_scalar(vox1[:], cnt_rb[:, 0:1], 1.0 / NVOX, None, ALU.mult)
        nc.tensor.matmul(out=vox_pb[:], lhsT=ones_row[:], rhs=vox1[:], start=True, stop=True)
        nc.scalar.activation(
            out=vox_row[:], in_=vox_pb[:].to_broadcast([128, B]), func=ACT.Copy
        )

    # ---------------- main stream: one output tile per time tile ----------------
    for j in range(N_TILES):
        js = slice(j * 128, (j + 1) * 128)
        if j < n_chunks:
            emit_grid_mms(j)
        if j == n_chunks:
            emit_vox_tail()
        ob = outp.tile([128, B, C_OUT], F32, tag="ob")
        for b in range(B):
            pv = psumv.tile([128, CV], F32, tag="pv")
            nc.tensor.matmul(
                out=pv[:], lhsT=wv_sb[:, js], rhs=vh_sb[:, b, :], start=True, stop=True
            )
            # split the PSUM->SBUF copies between DVE and ACT
            if b % 2 == 0:
                nc.vector.tensor_copy(out=ob[:, b, 0:CV], in_=pv[:])
            else:
                nc.scalar.activation(out=ob[:, b, 0:CV], in_=pv[:], func=ACT.Copy)
        nc.vector.tensor_copy(out=ob[:, :, CV:CV + CP], in_=pp_tiles[j][:])
        nc.vector.tensor_copy(out=ob[:, :, 544:550], in_=imu_sb[:, j, :, :])
        eng = nc.sync if j % 2 == 0 else nc.scalar
        if j >= J0:
            nc.vector.tensor_copy(
                out=ob[:, :, 550], in_=vox_row[:]
            )
            eng.dma_start(
                out=out[:, js, :].rearrange("b p c -> p b c"), in_=ob[:]
            )
        else:
            eng.dma_start(
                out=out[:, js, 0:550].rearrange("b p c -> p b c"), in_=ob[:, :, 0:550]
            )

    # vox column patches for the early tiles
    for j in range(J0):
        js = slice(j * 128, (j + 1) * 128)
        eng = nc.sync if j % 2 == 0 else nc.scalar
        eng.dma_start(
            out=out[:, js, 550:551].rearrange("b p o -> p (b o)"), in_=vox_row[:]
        )


_CACHE: dict[str, object] = {}


def _get_nc() -> bass.Bass:
    if "nc" not in _CACHE:
        from contextlib import ExitStack

        # Bacc (not plain Bass): its finalize() legalizes sync waits (HW
        # allows at most one wait per instruction; extras are split into
        # event-semaphore instructions).
        nc = bacc.Bacc(None, num_devices=N_CORES)
        with ExitStack() as ctx:
            tc = ctx.enter_context(tile.TileContext(nc))
            _emit(nc, tc, ctx)
        if not nc.is_finalized():
            nc.finalize()
        _CACHE["nc"] = nc
    return _CACHE["nc"]  # type: ignore[return-value]


def _run(inputs: dict, trace: bool = False):
    vision = np.asarray(inputs["vision"], dtype=np.float32)
    proprio = np.asarray(inputs["proprio"], dtype=np.float32)
    imu = np.asarray(inputs["imu"], dtype=np.float32)
    points = np.asarray(inputs["points"], dtype=np.float32)[:NPTS]
    # pad the point list with copies of point 0: duplicates never change
    # the occupancy count
    pts_pad = np.concatenate(
        [points, np.broadcast_to(points[0], (NPTS_PAD - NPTS, 3))], axis=0
    )
    pts_pad = np.ascontiguousarray(pts_pad)
    wv16 = _interp_weights_T(LV).astype(np.float16)
    wp16 = _interp_weights_T(LP).astype(np.float16)

    nc = _get_nc()
    in_maps = []
    for i in range(N_CORES):
        sl = slice(i * B, (i + 1) * B)
        in_maps.append({
            "vis": np.ascontiguousarray(
                vision[sl].transpose(1, 0, 2).astype(np.float16)),
            "pro": np.ascontiguousarray(
                proprio[sl].transpose(1, 0, 2).astype(np.float16)),
            "imu": np.ascontiguousarray(imu[sl].transpose(1, 0, 2)),
            "pts": pts_pad,
            "wv": wv16,
            "wp": wp16,
            "slab": np.full((128, 1), i * SLAB, dtype=np.int32),
        })
    res = run_bass_kernel_spmd(nc, in_maps, list(range(N_CORES)), trace=trace)
    full = np.concatenate([res.results[i]["out"] for i in range(N_CORES)], axis=0)
    return full, res


def kernel(**inputs) -> np.ndarray:
    full, _ = _run(inputs)
    return full


# revision 9
# speedup vs baseline: 1.0597x; 1.0597x over previous
"""Trainium2 Bass kernel for the BaselinePreprocessor problem.

Computes, for full inputs:
  fused = concat([interp(vision->T), interp(proprio->T), imu], -1)  # [64,1024,550]
  vox_mean = mean(occupancy grid 64^3 of 10k points)               # scalar
  out = concat([fused, vox_mean bcast], -1)                        # [64,1024,551]

Strategy (v3):
- Pure data parallel over batch (8 cores x 8 batches).
- Interp along time is a dense fp16 TensorE matmul with host-precomputed
  weights (one matmul per time tile, tolerance 2e-2 >> fp16 error ~1e-3).
- Inputs are host-transposed/cast so every DMA has large contiguous
  descriptors (imu -> [T,B,6] gives 192B rows instead of 24B).
- Output written as ONE DMA per time tile covering all 8 batches
  ([128, 8, 550/551] -> 2200B descriptors), alternating sync/scalar queues.
- Voxel occupancy without DRAM scatter: points are replicated to all
  cores; core i counts distinct voxels in slab [i*32768,(i+1)*32768) via
  bf16 one-hot is_equal tiles + 79 accumulating matmuls into one PSUM
  tile (count[hi,lo] += [hi(idx_k)==hi]*[lo(idx_k)==lo]), then a Sign
  activation with accum_out clamps+row-sums, and a tiny AllReduce(add)
  sums the disjoint slab counts. The voxel chain is emitted FIRST so the
  collective's ~40us fixed latency overlaps the output stream; tiles
  j >= J0 get the vox column inline, earlier tiles get column patch DMAs.
"""

import numpy as np

import concourse.bacc as bacc
import concourse.bass as bass
import concourse.mybir as mybir
import concourse.tile as tile
from concourse.bass_utils import run_bass_kernel_spmd

F32 = mybir.dt.float32
F16 = mybir.dt.float16
BF16 = mybir.dt.bfloat16
I32 = mybir.dt.int32
ALU = mybir.AluOpType
ACT = mybir.ActivationFunctionType

N_CORES = 8
B = 8                      # batches per core
T = 1024
LV, CV = 64, 512           # vision input time-len, channels
LP, CP = 256, 32           # proprio
CI = 6                     # imu channels (identity interp: L == T)
C_OUT = 551
GRID = 64
NVOX = GRID * GRID * GRID  # 262144
NPTS = 10000
PTS_F = 79                 # points laid out [128, 79] (padded to 10112)
NPTS_PAD = 128 * PTS_F
SLAB = NVOX // N_CORES     # 32768 voxels per core's slab
HI, LO = 128, 256          # slab voxel index split: idx_local = hi*256 + lo
N_TILES = T // 128         # 8 time tiles of 128 rows
OH_CHUNKS = [20, 20, 20, 19]  # point-column chunking for one-hot builds
J0 = 6                     # tiles >= J0 write the vox column inline


def _interp_weights_T(L: int) -> np.ndarray:
    """W^T [L, T] with W the [T, L] linear-interp matrix (align_corners)."""
    scale = np.float32((L - 1) / (T - 1))
    pos = np.arange(T, dtype=np.float32) * scale
    lo = np.clip(np.floor(pos).astype(np.int32), 0, L - 1)
    hi = np.minimum(lo + 1, L - 1)
    w = (pos - lo.astype(np.float32)).astype(np.float32)
    wt = np.zeros((L, T), dtype=np.float32)
    np.add.at(wt, (lo, np.arange(T)), np.float32(1.0) - w)
    np.add.at(wt, (hi, np.arange(T)), w)
    return np.ascontiguousarray(wt)


def _proprio_chunks_needed(j: int) -> list[int]:
    """Which K=128 row chunks of W_p^T have nonzeros for time tile j."""
    lo0 = (128 * j * (LP - 1)) // (T - 1)
    lo1 = (128 * j + 127) * (LP - 1) // (T - 1)
    hi1 = min(lo1 + 1, LP - 1)
    ks = []
    if lo0 < 128:
        ks.append(0)
    if hi1 >= 128:
        ks.append(1)
    return ks


def _emit(nc: bass.Bass, tc: tile.TileContext, ctx):
    vis = nc.declare_dram_parameter("vis", [LV, B, CV], F16, isOutput=False)
    pro = nc.declare_dram_parameter("pro", [LP, B, CP], F16, isOutput=False)
    imu = nc.declare_dram_parameter("imu", [T, B, CI], F32, isOutput=False)
    pts = nc.declare_dram_parameter("pts", [NPTS_PAD, 3], F32, isOutput=False)
    wv = nc.declare_dram_parameter("wv", [LV, T], F16, isOutput=False)
    wp = nc.declare_dram_parameter("wp", [LP, T], F16, isOutput=False)
    slab = nc.declare_dram_parameter("slab", [128, 1], I32, isOutput=False)
    out = nc.declare_dram_parameter("out", [B, T, C_OUT], F32, isOutput=True)

    cnt_dram = nc.dram_tensor("cnt", [1, 128], F32)
    cnt_sh = nc.dram_tensor("cnt_sh", [1, 128], F32, addr_space="Shared")

    const = ctx.enter_context(tc.tile_pool(name="const", bufs=1))
    vxw = ctx.enter_context(tc.tile_pool(name="vxw", bufs=1))
    ohp = ctx.enter_context(tc.tile_pool(name="ohp", bufs=4))
    rp = ctx.enter_context(tc.tile_pool(name="rp", bufs=4))
    outp = ctx.enter_context(tc.tile_pool(name="outp", bufs=3))
    psumv = ctx.enter_context(tc.tile_pool(name="psumv", bufs=3, space="PSUM"))
    psump = ctx.enter_context(tc.tile_pool(name="psump", bufs=3, space="PSUM"))
    psumg = ctx.enter_context(tc.tile_pool(name="psumg", bufs=1, space="PSUM"))
    psums = ctx.enter_context(tc.tile_pool(name="psums", bufs=1, space="PSUM"))

    # ---------------- loads (pts first: the voxel chain leads) ----------------
    pts_sb = vxw.tile([128, PTS_F, 3], F32)
    nc.sync.dma_start(out=pts_sb[:], in_=pts[:].rearrange("(p f) c -> p f c", p=128))
    slab_sb = vxw.tile([128, 1, 1], I32)
    nc.sync.dma_start(out=slab_sb[:, :, 0], in_=slab[:])
    wv_sb = const.tile([LV, T], F16)
    nc.scalar.dma_start(out=wv_sb[:], in_=wv[:])
    wp_sb = const.tile([128, 2, T], F16)
    nc.scalar.dma_start(out=wp_sb[:], in_=wp[:].rearrange("(k p) t -> p k t", p=128))
    vh_sb = const.tile([LV, B, CV], F16)
    nc.scalar.dma_start(out=vh_sb[:], in_=vis[:])
    pro_sb = const.tile([128, 2, B, CP], F16)
    nc.scalar.dma_start(out=pro_sb[:], in_=pro[:].rearrange("(k p) b c -> p k b c", p=128))
    imu_sb = const.tile([128, N_TILES, B, CI], F32)
    nc.scalar.dma_start(out=imu_sb[:], in_=imu[:].rearrange("(j p) b c -> p j b c", p=128))

    # iotas for the one-hot builds (bf16: integers <= 256 are exact)
    iota_hi_i = const.tile([128, 1, HI], I32)
    nc.gpsimd.iota(iota_hi_i[:], pattern=[[1, HI]], base=0, channel_multiplier=0)
    iota_lo_i = const.tile([128, 1, LO], I32)
    nc.gpsimd.iota(iota_lo_i[:], pattern=[[1, LO]], base=0, channel_multiplier=0)
    iota_hi = const.tile([128, 1, HI], BF16)
    nc.vector.tensor_copy(out=iota_hi[:], in_=iota_hi_i[:])
    iota_lo = const.tile([128, 1, LO], BF16)
    nc.vector.tensor_copy(out=iota_lo[:], in_=iota_lo_i[:])
    ones_col = const.tile([128, 1], F32)
    nc.gpsimd.memset(ones_col[:], 1.0)
    ones_row = const.tile([1, 128], F32)
    nc.gpsimd.memset(ones_row[:], 1.0)

    # ---------------- voxel index math (vector, leads the queue) ----------------
    # q_c = clip(trunc((p_c + 2) * 16), 0, 63), computed clip-then-floor
    # (equivalent: trunc==floor on the surviving non-negative range).
    # floor via int32 round-trip (any rounding mode) + is_gt correction.
    q = []
    ji = vxw.tile([128, PTS_F, 1], I32)
    gt = vxw.tile([128, PTS_F, 1], F32)
    for c in range(3):
        qc = vxw.tile([128, PTS_F, 1], F32, tag=f"q{c}")
        nc.vector.tensor_scalar(qc[:, :, 0], pts_sb[:, :, c], 2.0, 16.0, ALU.add, ALU.mult)
        nc.vector.tensor_scalar(qc[:], qc[:], 63.0, 0.0, ALU.min, ALU.max)
        rt = vxw.tile([128, PTS_F, 1], F32, tag=f"rt{c}")
        nc.vector.tensor_copy(out=ji[:], in_=qc[:])
        nc.vector.tensor_copy(out=rt[:], in_=ji[:])
        nc.vector.tensor_tensor(gt[:], rt[:], qc[:], ALU.is_gt)
        nc.vector.tensor_tensor(qc[:], rt[:], gt[:], ALU.subtract)
        q.append(qc)
    acc = vxw.tile([128, PTS_F, 1], F32)
    nc.vector.tensor_scalar(acc[:], q[0][:], 64.0, None, ALU.mult)
    nc.vector.tensor_tensor(acc[:], acc[:], q[1][:], ALU.add)
    nc.vector.tensor_scalar(acc[:], acc[:], 64.0, None, ALU.mult)
    nc.vector.tensor_tensor(acc[:], acc[:], q[2][:], ALU.add)
    idx_i = vxw.tile([128, PTS_F, 1], I32)
    nc.vector.tensor_copy(out=idx_i[:], in_=acc[:])  # exact integers -> exact
    # slab-local index; out-of-slab points self-mask (hi outside [0,128))
    nc.vector.tensor_tensor(
        idx_i[:], idx_i[:], slab_sb[:].to_broadcast([128, PTS_F, 1]), ALU.subtract
    )
    hi_i = vxw.tile([128, PTS_F, 1], I32)
    nc.vector.tensor_scalar(hi_i[:], idx_i[:], 8, None, ALU.arith_shift_right)
    lo_i = vxw.tile([128, PTS_F, 1], I32)
    nc.vector.tensor_scalar(lo_i[:], idx_i[:], 255, None, ALU.bitwise_and)
    # bf16 copies: slab-local hi in [0,128) / lo in [0,256) are exact;
    # out-of-slab hi only needs to stay outside [0,128), which rounding
    # preserves (relative error << distance to the valid range).
    hi_b = vxw.tile([128, PTS_F, 1], BF16)
    nc.vector.tensor_copy(out=hi_b[:], in_=hi_i[:])
    lo_b = vxw.tile([128, PTS_F, 1], BF16)
    nc.vector.tensor_copy(out=lo_b[:], in_=lo_i[:])

    # one-hot chunks (vector, right behind the index math)
    grid_ps = psumg.tile([128, LO], F32)
    n_chunks = len(OH_CHUNKS)
    chunk_start = [sum(OH_CHUNKS[:c]) for c in range(n_chunks)]
    oh_tiles = {}
    r_tiles = {}
    for c in range(n_chunks):
        f0, fn = chunk_start[c], OH_CHUNKS[c]
        oh = ohp.tile([128, max(OH_CHUNKS), HI], BF16, tag="oh")
        nc.vector.tensor_tensor(
            oh[:, 0:fn, :],
            hi_b[:, f0:f0 + fn, :].to_broadcast([128, fn, HI]),
            iota_hi[:].to_broadcast([128, fn, HI]),
            ALU.is_equal,
        )
        r = rp.tile([128, max(OH_CHUNKS), LO], BF16, tag="r")
        nc.vector.tensor_tensor(
            r[:, 0:fn, :],
            lo_b[:, f0:f0 + fn, :].to_broadcast([128, fn, LO]),
            iota_lo[:].to_broadcast([128, fn, LO]),
            ALU.is_equal,
        )
        oh_tiles[c] = oh
        r_tiles[c] = r

    def emit_grid_mms(c):
        f0, fn = chunk_start[c], OH_CHUNKS[c]
        for f in range(fn):
            nc.tensor.matmul(
                out=grid_ps[:],
                lhsT=oh_tiles[c][:, f, :],
                rhs=r_tiles[c][:, f, :],
                start=(f0 + f == 0),
                stop=(f0 + f == NPTS_PAD // 128 - 1),
            )

    # ---------------- proprio prepass pieces (interleaved below) ----------------
    pp_tiles = [
        const.tile([128, B, CP], F32, tag=f"ppsb{j}", name=f"ppsb{j}")
        for j in range(N_TILES)
    ]

    def emit_pp(j):
        ppj = psump.tile([128, B, CP], F32, tag="pp")
        js = slice(j * 128, (j + 1) * 128)
        ks = _proprio_chunks_needed(j)
        for i, k in enumerate(ks):
            nc.tensor.matmul(
                out=ppj[:],
                lhsT=wp_sb[:, k, js],
                rhs=pro_sb[:, k, :, :],
                start=(i == 0),
                stop=(i == len(ks) - 1),
            )
        # ACT copies PSUM->SBUF; keeps DVE free for the stream
        nc.scalar.activation(out=pp_tiles[j][:], in_=ppj[:], func=ACT.Copy)

    # ---------------- voxel count -> AllReduce pieces ----------------
    occ = vxw.tile([128, LO], BF16)
    red = vxw.tile([128, 1], F32)
    cnt_sb = vxw.tile([1, 128], F32)
    nc.gpsimd.memset(cnt_sb[:], 0.0)
    cnt_ps = psums.tile([1, 1], F32, tag="cnt")
    cnt_rb = vxw.tile([1, 128], F32)
    vox1 = vxw.tile([1, 1], F32)
    vox_row = vxw.tile([128, B], F32)

    def emit_vox_count():
        # Sign(count): counts >= 0 -> exactly the 0/1 occupancy; accum_out row-sums
        nc.scalar.activation(out=occ[:], in_=grid_ps[:], func=ACT.Sign, accum_out=red[:])
        nc.tensor.matmul(out=cnt_ps[:], lhsT=red[:], rhs=ones_col[:], start=True, stop=True)
        nc.scalar.activation(out=cnt_sb[:, 0:1], in_=cnt_ps[:], func=ACT.Copy)
        nc.gpsimd.dma_start(out=cnt_dram[:], in_=cnt_sb[:])
        nc.gpsimd.collective_compute(
            "AllReduce",
            ALU.add,
            replica_groups=[list(range(N_CORES))],
            ins=[cnt_dram[:]],
            outs=[cnt_sh[:]],
        )
        nc.gpsimd.dma_start(out=cnt_rb[:], in_=cnt_sh[:])
        nc.gpsimd.tensor_scalar(vox1[:], cnt_rb[:, 0:1], 1.0 / NVOX, None, ALU.mult)

    def emit_vox_bcast():
        vox_pb = psumv.tile([128, CV], F32, tag="pv")
        nc.tensor.matmul(
            out=vox_pb[:, 0:1], lhsT=ones_row[:], rhs=vox1[:], start=True, stop=True
        )
        nc.scalar.activation(
            out=vox_row[:], in_=vox_pb[:, 0:1].to_broadcast([128, B]), func=ACT.Copy
        )

    # tensor-queue interleave: keep vision matmuls flowing while the grid
    # matmul chunks and pp matmuls slot into the gaps
    pp_schedule = {0: [0, 1, 2], 1: [3, 4], 2: [5], 3: [6, 7]}

    # ---------------- main stream: one output tile per time tile ----------------
    for j in range(N_TILES):
        js = slice(j * 128, (j + 1) * 128)
        for pj in pp_schedule.get(j, []):
            emit_pp(pj)
        ob = outp.tile([128, B, C_OUT], F32, tag="ob")
        for b in range(B):
            pv = psumv.tile([128, CV], F32, tag="pv")
            nc.tensor.matmul(
                out=pv[:], lhsT=wv_sb[:, js], rhs=vh_sb[:, b, :], start=True, stop=True
            )
            # split the PSUM->SBUF copies between DVE and ACT
            if b % 2 == 0:
                nc.vector.tensor_copy(out=ob[:, b, 0:CV], in_=pv[:])
            else:
                nc.scalar.activation(out=ob[:, b, 0:CV], in_=pv[:], func=ACT.Copy)
        nc.vector.tensor_copy(out=ob[:, :, CV:CV + CP], in_=pp_tiles[j][:])
        nc.vector.tensor_copy(out=ob[:, :, 544:550], in_=imu_sb[:, j, :, :])
        if j < n_chunks:
            emit_grid_mms(j)
        if j == n_chunks:
            emit_vox_count()
        eng = nc.sync if j % 2 == 0 else nc.scalar
        if j >= J0:
            if j == J0:
                emit_vox_bcast()
            nc.scalar.activation(out=ob[:, :, 550], in_=vox_row[:], func=ACT.Copy)
            eng.dma_start(
                out=out[:, js, :].rearrange("b p c -> p b c"), in_=ob[:]
            )
        else:
            eng.dma_start(
                out=out[:, js, 0:550].rearrange("b p c -> p b c"), in_=ob[:, :, 0:550]
            )

    # vox column patches for the early tiles
    for j in range(J0):
        js = slice(j * 128, (j + 1) * 128)
        eng = nc.sync if j % 2 == 0 else nc.scalar
        eng.dma_start(
            out=out[:, js, 550:551].rearrange("b p o -> p (b o)"), in_=vox_row[:]
        )


_CACHE: dict[str, object] = {}


def _get_nc() -> bass.Bass:
    if "nc" not in _CACHE:
        from contextlib import ExitStack

        # Bacc (not plain Bass): its finalize() legalizes sync waits (HW
        # allows at most one wait per instruction; extras are split into
        # event-semaphore instructions).
        nc = bacc.Bacc(None, num_devices=N_CORES)
        with ExitStack() as ctx:
            tc = ctx.enter_context(tile.TileContext(nc))
            _emit(nc, tc, ctx)
        if not nc.is_finalized():
            nc.finalize()
        _CACHE["nc"] = nc
    return _CACHE["nc"]  # type: ignore[return-value]


def _run(inputs: dict, trace: bool = False):
    vision = np.asarray(inputs["vision"], dtype=np.float32)
    proprio = np.asarray(inputs["proprio"], dtype=np.float32)
    imu = np.asarray(inputs["imu"], dtype=np.float32)
    points = np.asarray(inputs["points"], dtype=np.float32)[:NPTS]
    # pad the point list with copies of point 0: duplicates never change
    # the occupancy count
    pts_pad = np.concatenate(
        [points, np.broadcast_to(points[0], (NPTS_PAD - NPTS, 3))], axis=0
    )
    pts_pad = np.ascontiguousarray(pts_pad)
    wv16 = _interp_weights_T(LV).astype(np.float16)
    wp16 = _interp_weights_T(LP).astype(np.float16)

    nc = _get_nc()
    in_maps = []
    for i in range(N_CORES):
        sl = slice(i * B, (i + 1) * B)
        in_maps.append({
            "vis": np.ascontiguousarray(
                vision[sl].transpose(1, 0, 2).astype(np.float16)),
            "pro": np.ascontiguousarray(
                proprio[sl].transpose(1, 0, 2).astype(np.float16)),
            "imu": np.ascontiguousarray(imu[sl].transpose(1, 0, 2)),
            "pts": pts_pad,
            "wv": wv16,
            "wp": wp16,
            "slab": np.full((128, 1), i * SLAB, dtype=np.int32),
        })
    res = run_bass_kernel_spmd(nc, in_maps, list(range(N_CORES)), trace=trace)
    full = np.concatenate([res.results[i]["out"] for i in range(N_CORES)], axis=0)
    return full, res


def kernel(**inputs) -> np.ndarray:
    full, _ = _run(inputs)
    return full
